# revision 3
# baseline (speedup 1.0000x reference)
"""Trainium2 Bass kernel v2 for 2-layer GAT + MLP (nn_GAT_61263413510492).

Diagonal scheme: nodes sorted by in-degree, 128-node dst tiles; within a
tile, partition p == dst slot p for EVERY edge slot (diag). Per dst-tile,
edges live in B blocks of 128 slots; block b, partition p holds one of
dst p's in-edges (or a poison dummy). Aggregation = identity-lhsT matmuls
accumulating over blocks in PSUM, with the softmax denominator riding as
4 extra rhs columns (the per-edge ex values). a_d is per-partition (the
tile's own 128 dsts) - no gather, no selector builds.

Edges whose dst overflows its per-partition slot capacity go to "repair"
blocks (still diagonal). int16 gather indices are handled by splitting
the table at row 32768: lo-edges occupy the leading blocks (gathered from
table[:32768]), hi-edges the trailing blocks (from table[32768:]).
"""

import os

import numpy as np

import concourse.bass as bass
import concourse.bacc as bacc
import concourse.mybir as mybir
import concourse.tile as tile
from concourse.bass_utils import run_bass_kernel_spmd

f32 = mybir.dt.float32
f16 = mybir.dt.float16
i16 = mybir.dt.int16
AF = mybir.ActivationFunctionType
ALU = mybir.AluOpType

NEG_SLOPE = 0.2
EPS = 1e-16
POISON = -80.0

N = 50000
E = 800000
IN_DIM = 128
HID = 64
HEADS = 4
OUT_DIM = 121
HC = HEADS * HID          # 256
HCA = HC + 8              # 264 (h, a_s, a_d)
HROW = 384                # gather row: h(256) | a_s(4) | pad -> 768B
NC = 8
NT = 49                   # dst tiles per core
NPC = NT * 128            # 6272 rows per core
NP = NPC * NC             # 50176
SPLIT = 32768             # table split for int16 gather indices
CHUNKS = [13, 12, 12, 12]  # AllGather slot-chunks (sum == NT)
MAXIDX = 1024             # idxs per dma_gather call (single-packet limit)


# ---------------------------------------------------------------------------
# Host-side prep
# ---------------------------------------------------------------------------

def host_prep(edge_index):
    src = np.concatenate([np.asarray(edge_index[0], np.int64), np.arange(N)])
    dst = np.concatenate([np.asarray(edge_index[1], np.int64), np.arange(N)])
    indeg = np.bincount(dst, minlength=N)

    # Table rows are laid out in AG-chunk-major order so AllGathers can be
    # chunked (each chunk's 8-core concat is contiguous):
    #   row(c, j, p) = gb[k] + c*rows_k + (j - cstart_k)*128 + p,
    # where k = chunk of slot j.
    cstart = np.cumsum([0] + CHUNKS).tolist()
    rows_k = [CHUNKS[k] * 128 for k in range(len(CHUNKS))]
    gb = np.cumsum([0] + [NC * r for r in rows_k]).tolist()

    def row_of(c, j, p):
        k = 0
        while j >= cstart[k + 1]:
            k += 1
        return gb[k] + c * rows_k[k] + (j - cstart[k]) * 128 + p

    # reserved poison cells: (c, 24, 127) lo-side, (c, NT-1, 127) hi-side
    reserved = set()
    for c in range(NC):
        reserved.add(row_of(c, cstart[2] - 1, 127))
        reserved.add(row_of(c, NT - 1, 127))
    cells = []
    core_of_row = np.zeros(NP, np.int64)
    slot_of_row = np.zeros(NP, np.int64)
    part_of_row = np.zeros(NP, np.int64)
    for t in range(NP // 128):
        c, j = t % NC, t // NC
        for p in range(128):
            r = row_of(c, j, p)
            core_of_row[r] = c
            slot_of_row[r] = j
            part_of_row[r] = p
            if r not in reserved:
                cells.append(r)
    cells = np.array(cells, np.int64)
    order = np.argsort(-indeg, kind="stable")
    new_of_old = np.full(N, -1, np.int64)
    new_of_old[order] = cells[:N]
    old_of_new = np.full(NP, -1, np.int64)
    old_of_new[new_of_old] = np.arange(N)

    s_new = new_of_old[src]
    d_new = new_of_old[dst]

    # per (core, slot) tile: edge lists
    core_of = core_of_row[d_new]
    slot_of = slot_of_row[d_new]
    part_of = part_of_row[d_new]
    islo = s_new < SPLIT

    # group edges by (core, slot)
    key = core_of * NT + slot_of
    eorder = np.argsort(key, kind="stable")
    s_e = s_new[eorder]
    p_e = part_of[eorder]
    lo_e = islo[eorder]
    starts = np.zeros(NC * NT + 1, np.int64)
    np.cumsum(np.bincount(key, minlength=NC * NT), out=starts[1:])

    PO_LO = row_of(0, cstart[2] - 1, 127)
    assert PO_LO < SPLIT
    PO_HI_ROW = row_of(NC - 1, NT - 1, 127)
    assert PO_HI_ROW >= SPLIT
    PO_HI = PO_HI_ROW - SPLIT

    # First pass: per tile, per dst, split lo/hi and count
    tile_edges = {}
    for c in range(NC):
        for j in range(NT):
            a, b = starts[c * NT + j], starts[c * NT + j + 1]
            tile_edges[(c, j)] = (s_e[a:b], p_e[a:b], lo_e[a:b])

    # per slot-group j: BL/BH (main diag blocks) and RL/RH (permuted repair
    # blocks), shared across cores for SPMD uniformity. Repair blocks hold
    # the per-dst overflow at arbitrary partitions (off-diagonal) with at
    # most one edge per dst AND per partition per block; a tiny selector
    # matmul routes them.
    BLs, BHs, RLs, RHs = [], [], [], []
    per_tile = {}
    for j in range(NT):
        # per-dst class degree counts for all 8 cores of this slot group
        Lcnt, Hcnt = [], []
        for c in range(NC):
            s_t, p_t, l_t = tile_edges[(c, j)]
            Lcnt.append(np.bincount(p_t[l_t], minlength=128))
            Hcnt.append(np.bincount(p_t[~l_t], minlength=128))

        def best_B(cnts):
            hi0 = max(1, -(-max(int(cn.sum()) for cn in cnts) // 128))
            best, bestv = hi0, None
            for Bm in range(hi0, max(0, hi0 - 8), -1):
                rep = max(-(-int(np.maximum(cn - Bm, 0).sum()) // 128)
                          for cn in cnts)
                tot = Bm + rep
                if bestv is None or tot < bestv:
                    best, bestv = Bm, tot
            return best
        BL = best_B(Lcnt)
        BH = best_B(Hcnt)
        RLj = RHj = 0
        plans = []
        for c in range(NC):
            s_t, p_t, l_t = tile_edges[(c, j)]
            plan = {}
            rep_lo, rep_hi = [], []
            for p in range(128):
                m = p_t == p
                sl = s_t[m & l_t]
                sh = s_t[m & ~l_t]
                plan[p] = (sl[:BL], sh[:BH])
                for k, sv in enumerate(sl[BL:]):
                    rep_lo.append((p, sv, k))   # (dst slot, src, ordinal)
                for k, sv in enumerate(sh[BH:]):
                    rep_hi.append((p, sv, k))
            # bin-pack repair edges densely: repeats of the same dst within
            # a block are fine (the P_r one-hot handles them); the AD side
            # uses a PE transpose of P_r, so no per-dst uniqueness needed.
            def pack(rep):
                nblk = -(-len(rep) // 128)
                blocks = [[] for _ in range(nblk)]
                for i, (d, sv, k) in enumerate(rep):
                    blocks[i // 128].append((d, sv))
                return blocks
            plan["rep_lo"] = pack(rep_lo)
            plan["rep_hi"] = pack(rep_hi)
            RLj = max(RLj, len(plan["rep_lo"]))
            RHj = max(RHj, len(plan["rep_hi"]))
            plans.append(plan)
        BLs.append(BL)
        BHs.append(BH)
        RLs.append(RLj)
        RHs.append(RHj)
        for c in range(NC):
            per_tile[(c, j)] = plans[c]

    # build per-core idx arrays; block layout per tile:
    #   [lo-main BL | lo-repair RL | hi-main BH | hi-repair RH]
    nlo_blk = [BLs[j] + RLs[j] for j in range(NT)]
    nhi_blk = [BHs[j] + RHs[j] for j in range(NT)]
    Bj = [nlo_blk[j] + nhi_blk[j] for j in range(NT)]

    idx_lo = []   # per core: flat int16 array, concat over j
    idx_hi = []
    # per-core repair selector tables: dst-slot per partition (dstloc) and
    # partition per dst (invpos), one column per repair block.
    NREP = sum(RLs) + sum(RHs)
    dstloc_t = [np.full((128, NREP), 255.0, np.float32) for _ in range(NC)]
    invpos_t = [np.full((128, NREP), 255.0, np.float32) for _ in range(NC)]
    for c in range(NC):
        alo, ahi = [], []
        rcol = 0
        for j in range(NT):
            BL, BH, RL, RH = BLs[j], BHs[j], RLs[j], RHs[j]
            plan = per_tile[(c, j)]
            lo_grid = np.full((BL + RL, 128), PO_LO, np.int64)
            hi_grid = np.full((BH + RH, 128), PO_HI, np.int64)
            for p in range(128):
                sl, sh = plan[p]
                lo_grid[: len(sl), p] = sl
                hi_grid[: len(sh), p] = sh - SPLIT
            for k, blk in enumerate(plan["rep_lo"]):
                for q, (d, sv) in enumerate(blk):
                    lo_grid[BL + k, q] = sv
                    dstloc_t[c][q, rcol + k] = d
                    invpos_t[c][d, rcol + k] = q
            for k, blk in enumerate(plan["rep_hi"]):
                for q, (d, sv) in enumerate(blk):
                    hi_grid[BH + k, q] = sv - SPLIT
                    dstloc_t[c][q, rcol + RL + k] = d
                    invpos_t[c][d, rcol + RL + k] = q
            rcol += RL + RH
            alo.append(lo_grid.reshape(-1))
            ahi.append(hi_grid.reshape(-1))
        idx_lo.append(np.concatenate(alo).astype(np.int16))
        idx_hi.append(np.concatenate(ahi).astype(np.int16))

    def wrap16(flat):
        # idx number k -> (lane k%16, col k//16); replicate to 128 partitions
        w = flat.reshape(-1, 16).T               # [16, S]
        return np.ascontiguousarray(np.tile(w, (8, 1)))

    prep = dict(
        new_of_old=new_of_old, old_of_new=old_of_new,
        BLs=BLs, BHs=BHs, RLs=RLs, RHs=RHs, Bj=Bj,
        nlo_blk=nlo_blk, nhi_blk=nhi_blk,
        idx_lo=[wrap16(a) for a in idx_lo],
        idx_hi=[wrap16(a) for a in idx_hi],
        dstloc=dstloc_t, invpos=invpos_t, NREP=NREP,
        row_of=row_of, gb=gb, rows_k=rows_k, cstart=cstart,
        poison_rows=sorted(reserved),
        outperm=core_of_row * NPC + slot_of_row * 128 + part_of_row,
    )
    # host-built repair selectors: Psel[p, r*128+c] = (c == dstloc[p, r]),
    # PselT[d, r*128+c] = (dstloc[c, r] == d)
    iota = np.arange(128)
    psel, pselT = [], []
    for c in range(NC):
        dl = dstloc_t[c].astype(np.int64)          # [128, NREP]
        P = (iota[None, None, :] == dl.T[:, :, None])   # [NREP, 128p, 128c]
        PT = np.swapaxes(P, 1, 2)
        psel.append(np.ascontiguousarray(
            P.transpose(1, 0, 2).reshape(128, -1)).astype(np.float16))
        pselT.append(np.ascontiguousarray(
            PT.transpose(1, 0, 2).reshape(128, -1)).astype(np.float16))
    prep["psel"] = psel
    prep["pselT"] = pselT
    return prep


def fold_weights(W, att_src, att_dst):
    K = W.shape[0]
    Wr = W.reshape(K, HEADS, HID)
    w_as = np.einsum("khd,hd->kh", Wr, att_src)
    w_ad = np.einsum("khd,hd->kh", Wr, att_dst)
    return np.concatenate([W, w_as, w_ad], axis=1).astype(np.float32)


# ---------------------------------------------------------------------------
# Numpy emulator (validates prep + device algorithm)
# ---------------------------------------------------------------------------

def emulate(prep, x, W1a, W2a, b1, b2, Wm1, bm1, Wm2, bm2):
    oon = prep["old_of_new"]
    xp = np.zeros((NP, IN_DIM), np.float32)
    xp[oon >= 0] = x[oon[oon >= 0]]
    Bj = prep["Bj"]

    def layer(hin, Wa, bb):
        Haug = (hin @ Wa).astype(np.float32)          # [NP, 264]
        htab = Haug.astype(np.float16)                # h + a_s + a_d
        # poison rows: a_s of reserved rows
        for r in prep["poison_rows"]:
            htab[r, HC:HC + 4] = POISON
        out = np.zeros((NP, HC), np.float32)
        BLs, RLs, BHs, RHs = (prep["BLs"], prep["RLs"],
                              prep["BHs"], prep["RHs"])
        for c in range(NC):
            ilo = prep["idx_lo"][c]
            ihi = prep["idx_hi"][c]
            dstloc = prep["dstloc"][c]
            olo = ohi = 0
            rcol = 0
            for j in range(NT):
                nlo, nhi = prep["nlo_blk"][j], prep["nhi_blk"][j]
                BL, RL, BH, RH = BLs[j], RLs[j], BHs[j], RHs[j]
                B = Bj[j]
                r0 = prep["row_of"](c, j, 0)
                rows = slice(r0, r0 + 128)
                ad = Haug[rows, HC + 4:HC + 8].astype(np.float16)
                # gather G
                G = np.zeros((128, B, HROW), np.float16)
                AS = np.zeros((128, B, 4), np.float32)

                def fill(grid_idx, b0, nblk, base, off):
                    flat = grid_idx[:16, off:off + nblk * 8].T.reshape(-1)
                    k = np.arange(flat.size)
                    rowd = base + flat.astype(np.int64)
                    G[k % 128, b0 + k // 128, 0:HC] = htab[rowd][:, 0:HC]
                    AS[k % 128, b0 + k // 128] = htab[rowd][:, HC:HC + 4]

                fill(ilo, 0, nlo, 0, olo)
                fill(ihi, nlo, nhi, SPLIT, ohi)
                olo += nlo * 8
                ohi += nhi * 8
                # per-block dst routing + AD
                is_rep = np.zeros(B, bool)
                is_rep[BL:BL + RL] = True
                is_rep[nlo + BH:nlo + BH + RH] = True
                repcols = list(range(rcol, rcol + RL)) + \
                    list(range(rcol + RL, rcol + RL + RH))
                rcol += RL + RH
                ADb = np.zeros((128, B, 4), np.float32)
                route = np.tile(np.arange(128)[:, None], (1, B))
                ri = 0
                for b in range(B):
                    if not is_rep[b]:
                        ADb[:, b, :] = ad.astype(np.float32)
                    else:
                        dl = dstloc[:, repcols[ri]].astype(np.int64)
                        ri += 1
                        valid = dl < 128
                        ADb[valid, b, :] = ad[dl[valid]].astype(np.float32)
                        route[:, b] = np.where(valid, dl, -1)
                z = AS + ADb
                L = np.where(z > 0, z, NEG_SLOPE * z)
                EX = np.exp(L).astype(np.float16)
                Gs = (G[:, :, 0:HC].astype(np.float32)
                      * np.repeat(EX.astype(np.float32), HID, axis=2))
                agg = np.zeros((128, HC), np.float32)
                den = np.zeros((128, 4), np.float32)
                for b in range(B):
                    r = route[:, b]
                    v = r >= 0
                    np.add.at(agg, r[v], Gs[v, b])
                    np.add.at(den, r[v], EX[v, b].astype(np.float32))
                rec = 1.0 / (den + EPS)
                o = agg * np.repeat(rec, HID, axis=1) + bb[None, :]
                out[rows] = np.maximum(o, 0)
        return out.astype(np.float16).astype(np.float32)

    o1 = layer(xp, W1a, b1)
    o2 = layer(o1, W2a, b2)
    m = o2.astype(np.float16).astype(np.float32) @ Wm1 + bm1
    s = m.astype(np.float16).astype(np.float32) @ Wm2 + bm2
    out = 1.0 / (1.0 + np.exp(-s))
    res = np.zeros((N, OUT_DIM), np.float32)
    valid = oon >= 0
    res[oon[valid]] = out[valid]
    return res


# ---------------------------------------------------------------------------
# Bass program
# ---------------------------------------------------------------------------

def build_bass(prep):
    BLs, BHs = prep["BLs"], prep["BHs"]
    nlo_blk, nhi_blk, Bj = prep["nlo_blk"], prep["nhi_blk"], prep["Bj"]
    SLO = sum(nlo_blk) * 8     # idx cols (16-wrapped)
    SHI = sum(nhi_blk) * 8
    BMAX = max(Bj)

    CSTART = prep["cstart"]
    GB = prep["gb"]
    CS2 = CSTART[2]
    nc = bacc.Bacc("TRN2", num_devices=NC, num_swdge_queues=4,
                   dynamic_dma_scratch_size=16384)
    groups = [list(range(NC))]

    def inp(name, shape, dt):
        return nc.dram_tensor(name, list(shape), dt, kind="ExternalInput")

    xT = inp("xT", (IN_DIM, NPC), f32)
    w1a = inp("w1a", (IN_DIM, HCA), f32)
    w2a = inp("w2a", (128, 2 * HCA), f16)
    b1b = inp("b1b", (128, HC), f32)
    b2b = inp("b2b", (128, HC), f32)
    wm1 = inp("wm1", (128, 2 * HID), f16)
    bm1r = inp("bm1r", (1, HID), f16)
    wm2a = inp("wm2a", (HID + 1, OUT_DIM), f16)
    identf = inp("identf", (128, 128), f16)
    iotaf = inp("iotaf", (128, 128), f16)
    poim = inp("poim", (128, 4), f16)
    ones1 = inp("ones1", (1, 128), f16)
    idxlo = inp("idxlo", (128, SLO), i16)
    idxhi = inp("idxhi", (128, SHI), i16)
    NREP = prep["NREP"]
    psel = inp("psel", (128, NREP * 128), f16)
    pselT = inp("pselT", (128, NREP * 128), f16)

    out_ext = nc.dram_tensor("out", [NPC, OUT_DIM], f32, kind="ExternalOutput")

    def internal(name, shape, dt, shared=False):
        return nc.dram_tensor(
            name, list(shape), dt, kind="Internal",
            addr_space="Shared" if shared else "Local")

    h1chunk = internal("h1chunk", (NPC, HROW), f16)
    h2chunk = internal("h2chunk", (NPC, HROW), f16)
    h1tab = internal("h1tab", (NP, HROW), f16, shared=True)
    h2tab = internal("h2tab", (NP, HROW), f16, shared=True)

    with tile.TileContext(nc) as tc:
        with (
            tc.tile_pool(name="persist", bufs=1) as pp,
            tc.tile_pool(name="work", bufs=4) as wp,
            tc.tile_pool(name="gath", bufs=2) as gp,
            tc.tile_pool(name="gs", bufs=2) as gsp,
            tc.tile_pool(name="psA", bufs=3, space="PSUM") as psA,
            tc.tile_pool(name="psB", bufs=1, space="PSUM") as psB,
            tc.tile_pool(name="psT", bufs=2, space="PSUM") as psT,
        ):
            def load(apin, shape, dt, name):
                t = pp.tile(shape, dt, name=name, tag=name)
                nc.sync.dma_start(out=t[:], in_=apin[:])
                return t

            w1a_sb = load(w1a.ap(), [IN_DIM, HCA], f32, "w1a_sb")
            w2a_sb = load(w2a.ap(), [128, 2 * HCA], f16, "w2a_sb")
            b1b_sb = load(b1b.ap(), [128, HC], f32, "b1b_sb")
            b2b_sb = load(b2b.ap(), [128, HC], f32, "b2b_sb")
            wm1_sb = load(wm1.ap(), [128, 2 * HID], f16, "wm1_sb")
            bm1r_sb = load(bm1r.ap(), [1, HID], f16, "bm1r_sb")
            wm2a_sb = load(wm2a.ap(), [HID + 1, OUT_DIM], f16, "wm2a_sb")
            ident_sb = load(identf.ap(), [128, 128], f16, "ident_sb")
            iota_sb = load(iotaf.ap(), [128, 128], f16, "iota_sb")
            poim_sb = load(poim.ap(), [128, 4], f16, "poim_sb")
            ones1_sb = load(ones1.ap(), [1, 128], f16, "ones1_sb")

            idxlo_sb = load(idxlo.ap(), [128, SLO], i16, "idxlo_sb")
            idxhi_sb = load(idxhi.ap(), [128, SHI], i16, "idxhi_sb")

            ad_all = pp.tile([128, NT * 4], f16, name="ad_all", tag="ad_all")
            o1T = pp.tile([128, 2 * NPC], f16, name="o1T", tag="o1T")

            # ---------------- dense ----------------
            def dense(layer, j):
                ps = psA.tile([128, HCA], f32, tag="acc")
                if layer == 1:
                    xtj = wp.tile([IN_DIM, 128], f32, tag="xtj")
                    nc.sync.dma_start(out=xtj[:],
                                      in_=xT.ap()[:, bass.ts(j, 128)])
                    nc.tensor.matmul(
                        ps[:], lhsT=xtj[:],
                        rhs=w1a_sb[:], start=True, stop=True)
                else:
                    for half in range(2):
                        nc.tensor.matmul(
                            ps[:],
                            lhsT=o1T[:, half * NPC + j * 128:
                                     half * NPC + (j + 1) * 128],
                            rhs=w2a_sb[:, half * HCA:(half + 1) * HCA],
                            start=(half == 0), stop=(half == 1))
                hf = wp.tile([128, HROW], f16, tag="hf")
                nc.vector.tensor_copy(out=hf[:, 0:HC + 4], in_=ps[:, 0:HC + 4])
                nc.vector.memset(hf[:, HC + 4:HROW], 0.0)
                nc.vector.tensor_copy(
                    out=ad_all[:, j * 4:(j + 1) * 4],
                    in_=ps[:, HC + 4:HC + 8])
                if j in (CS2 - 1, NT - 1):
                    nc.vector.tensor_tensor(
                        out=hf[:, HC:HC + 4], in0=hf[:, HC:HC + 4],
                        in1=poim_sb[:], op=ALU.add)
                hchunk = h1chunk if layer == 1 else h2chunk
                nc.sync.dma_start(
                    out=hchunk.ap()[bass.ts(j, 128), :], in_=hf[:])

            def allgather(layer, k):
                hchunk = h1chunk if layer == 1 else h2chunk
                htab = h1tab if layer == 1 else h2tab
                cs, ck = CSTART[k], CHUNKS[k]
                if int(os.environ.get("WHOLEAG", "0")):
                    if k == len(CHUNKS) - 1:
                        nc.gpsimd.collective_compute(
                            "AllGather", ALU.bypass, replica_groups=groups,
                            ins=[hchunk.ap()], outs=[htab.ap()])
                    return
                nc.gpsimd.collective_compute(
                    "AllGather", ALU.bypass, replica_groups=groups,
                    ins=[hchunk.ap()[cs * 128:(cs + ck) * 128, :]],
                    outs=[htab.ap()[GB[k]:GB[k] + NC * ck * 128, :]])

            # ---------------- edge ----------------
            qctr = [0]

            def chunked_gather(G, b0, nblk, table_ap, idx_sb, idx_base):
                done = 0
                while done < nblk:
                    k = min(MAXIDX // 128, nblk - done)
                    nc.gpsimd.dma_gather(
                        out_ap=G[:, b0 + done:b0 + done + k, :],
                        in_ap=table_ap,
                        idxs_ap=idx_sb[:, idx_base + done * 8:
                                       idx_base + (done + k) * 8],
                        num_idxs=k * 128,
                        num_idxs_reg=k * 128,
                        elem_size=HROW,
                        queue_num=qctr[0] % 4)
                    qctr[0] += 1
                    done += k

            olo = [0]
            ohi = [0]
            roff = [0]

            def edge_prefetch(layer, j):
                htab = h1tab if layer == 1 else h2tab
                nlo, nhi = nlo_blk[j], nhi_blk[j]
                RT = prep["RLs"][j] + prep["RHs"][j]
                G = gp.tile([128, BMAX, HROW], f16, tag="G")
                chunked_gather(G, 0, nlo, htab.ap()[0:SPLIT, :],
                               idxlo_sb, olo[0])
                chunked_gather(G, nlo, nhi, htab.ap()[SPLIT:NP, :],
                               idxhi_sb, ohi[0])
                olo[0] += nlo * 8
                ohi[0] += nhi * 8
                sels = selsT = None
                if RT:
                    sels = wp.tile([128, RT, 128], f16, tag="sels")
                    nc.sync.dma_start(
                        out=sels[:],
                        in_=psel.ap()[:, roff[0] * 128:(roff[0] + RT) * 128])
                    selsT = wp.tile([128, RT, 128], f16, tag="selsT")
                    nc.sync.dma_start(
                        out=selsT[:],
                        in_=pselT.ap()[:, roff[0] * 128:(roff[0] + RT) * 128])
                    roff[0] += RT
                return (G, sels, selsT)

            def edge(layer, j, pf):
                bbias = b1b_sb if layer == 1 else b2b_sb
                B = Bj[j]
                nlo, nhi = nlo_blk[j], nhi_blk[j]
                BL, RL = BLs[j], prep["RLs"][j]
                BH, RH = BHs[j], prep["RHs"][j]
                RT = RL + RH
                G, sels, selsT = pf

                ad4 = ad_all[:, j * 4:(j + 1) * 4]
                if RT:
                    adps = psB.tile([128, RT * 4], f32, tag="adps")
                    for r in range(RT):
                        nc.tensor.matmul(
                            adps[:, r * 4:(r + 1) * 4], lhsT=selsT[:, r, :],
                            rhs=ad4, start=True, stop=True)

                aggf = psA.tile([128, HCA], f32, tag="acc")
                agg = aggf[:, 0:260]
                rep_of = {}
                for r in range(RL):
                    rep_of[BL + r] = r
                for r in range(RH):
                    rep_of[nlo + BH + r] = RL + r

                def half(b0, nb, mb0, mnb, rb0, rnb, radd):
                    # process blocks [b0, b0+nb): main range (mb0, mnb) uses
                    # ad4 broadcast, repair range (rb0, rnb) uses adps
                    z = wp.tile([128, nb, 4], f32, tag=f"z{b0 > 0}")
                    if mnb:
                        nc.vector.tensor_tensor(
                            out=z[:, mb0 - b0:mb0 - b0 + mnb, :],
                            in0=G[:, mb0:mb0 + mnb, HC:HC + 4],
                            in1=ad4.unsqueeze(1).broadcast_to([128, mnb, 4]),
                            op=ALU.add)
                    if rnb:
                        nc.vector.tensor_tensor(
                            out=z[:, rb0 - b0:rb0 - b0 + rnb, :],
                            in0=G[:, rb0:rb0 + rnb, HC:HC + 4],
                            in1=adps[:, radd * 4:(radd + rnb) * 4]
                            .rearrange("p (b h) -> p b h", h=4),
                            op=ALU.add)
                    zf = z[:].rearrange("p b h -> p (b h)")
                    lk = wp.tile([128, nb * 4], f32, tag=f"lk{b0 > 0}")
                    nc.vector.tensor_scalar(
                        out=lk[:], in0=zf, scalar1=NEG_SLOPE, scalar2=None,
                        op0=ALU.mult)
                    nc.vector.tensor_tensor(
                        out=lk[:], in0=lk[:], in1=zf, op=ALU.max)
                    EX = wp.tile([128, nb * 4], f16, tag=f"EX{b0 > 0}")
                    nc.scalar.activation(out=EX[:], in_=lk[:], func=AF.Exp)
                    EXP = gsp.tile([128, nb, 256], f16, tag=f"EXP{b0 > 0}")
                    nc.scalar.activation(
                        out=EXP[:].rearrange("p b (k s) -> p (b k) s", s=64),
                        in_=EX[:].unsqueeze(2).broadcast_to([128, nb * 4, 64]),
                        func=AF.Copy)
                    Gs = gsp.tile([128, nb, 260], f16, tag=f"Gs{b0 > 0}")
                    nc.vector.tensor_tensor(
                        out=Gs[:, :, 0:256], in0=G[:, b0:b0 + nb, 0:256],
                        in1=EXP[:], op=ALU.mult)
                    nc.vector.tensor_copy(
                        out=Gs[:, :, 256:260],
                        in_=EX[:].rearrange("p (b h) -> p b h", h=4))
                    for b in range(b0, b0 + nb):
                        lhsT = (sels[:, rep_of[b], :] if b in rep_of
                                else ident_sb[:])
                        nc.tensor.matmul(
                            agg, lhsT=lhsT, rhs=Gs[:, b - b0, :],
                            start=(b == 0), stop=(b == B - 1))

                half(0, nlo, 0, BL, BL, RL, 0)
                half(nlo, nhi, nlo, BH, nlo + BH, RH, RL)

                # epilogue: rec = 1/(den+eps); o = agg*rec + bias; relu
                rec = wp.tile([128, 4], f32, tag="rec")
                nc.vector.tensor_scalar(
                    out=rec[:], in0=aggf[:, 256:260], scalar1=EPS,
                    scalar2=None, op0=ALU.add)
                nc.vector.reciprocal(out=rec[:], in_=rec[:])
                o = wp.tile([128, HC], f32, tag="o")
                for h in range(HEADS):
                    nc.vector.scalar_tensor_tensor(
                        out=o[:, h * HID:(h + 1) * HID],
                        in0=aggf[:, h * HID:(h + 1) * HID],
                        scalar=rec[:, h:h + 1],
                        in1=bbias[:, h * HID:(h + 1) * HID],
                        op0=ALU.mult, op1=ALU.add)
                of = wp.tile([128, HC], f16, tag="of")
                nc.vector.tensor_scalar(
                    out=of[:], in0=o[:], scalar1=0.0, scalar2=None,
                    op0=ALU.max)

                if layer == 1:
                    for half in range(2):
                        tp = psT.tile([128, 128], f16, tag="tp")
                        nc.tensor.transpose(
                            tp[:], in_=of[:, bass.ts(half, 128)],
                            identity=ident_sb[:])
                        nc.vector.tensor_copy(
                            out=o1T[:, half * NPC + j * 128:
                                    half * NPC + (j + 1) * 128],
                            in_=tp[:])
                else:
                    mlp(j, of)

            # ---------------- MLP head ----------------
            def mlp(j, of):
                o2T = wp.tile([128, 2, 128], f16, tag="o2T")
                for half in range(2):
                    tp = psT.tile([128, 128], f16, tag="tp")
                    nc.tensor.transpose(
                        tp[:], in_=of[:, bass.ts(half, 128)],
                        identity=ident_sb[:])
                    nc.vector.tensor_copy(out=o2T[:, half, :], in_=tp[:])
                mps = psB.tile([128, HID], f32, tag="mps")
                for half in range(2):
                    nc.tensor.matmul(
                        mps[:], lhsT=o2T[:, half, :],
                        rhs=wm1_sb[:, half * HID:(half + 1) * HID],
                        start=(half == 0), stop=False)
                nc.tensor.matmul(
                    mps[:], lhsT=ones1_sb[:], rhs=bm1r_sb[:],
                    start=False, stop=True)
                m = wp.tile([128, HID + 1], f16, tag="m")
                nc.vector.memset(m[:, HID:HID + 1], 1.0)
                nc.vector.tensor_copy(out=m[:, 0:HID], in_=mps[:])
                mt = psT.tile([HID + 1, 128], f16, tag="tp")
                nc.tensor.transpose(mt[:], in_=m[:], identity=ident_sb[:])
                mtf = wp.tile([HID + 1, 128], f16, tag="mtf")
                nc.vector.tensor_copy(out=mtf[:], in_=mt[:])
                sps = psB.tile([128, OUT_DIM], f32, tag="sps")
                nc.tensor.matmul(
                    sps[:], lhsT=mtf[:], rhs=wm2a_sb[:], start=True, stop=True)
                # sigmoid(x) = 1/(1+exp(-x)) -- keep ACT on the Exp table
                en = wp.tile([128, OUT_DIM], f32, tag="en")
                nc.scalar.activation(out=en[:], in_=sps[:], func=AF.Exp,
                                     scale=-1.0)
                nc.vector.tensor_scalar(
                    out=en[:], in0=en[:], scalar1=1.0, scalar2=None,
                    op0=ALU.add)
                osig = wp.tile([128, OUT_DIM], f32, tag="osig")
                nc.vector.reciprocal(out=osig[:], in_=en[:])
                nc.sync.dma_start(
                    out=out_ext.ap()[bass.ts(j, 128), :], in_=osig[:])

            # ---------------- schedule ----------------
            for k in range(len(CHUNKS)):
                for j in range(CSTART[k], CSTART[k] + CHUNKS[k]):
                    dense(1, j)
                allgather(1, k)
            kd = 0
            pf = edge_prefetch(1, 0)
            for j in range(NT):
                nxt = edge_prefetch(1, j + 1) if j + 1 < NT else None
                edge(1, j, pf)
                pf = nxt
                dense(2, j)
                if kd < len(CHUNKS) and j == CSTART[kd] + CHUNKS[kd] - 1:
                    allgather(2, kd)
                    kd += 1
            olo[0] = 0
            ohi[0] = 0
            roff[0] = 0
            pf = edge_prefetch(2, 0)
            for j in range(NT):
                nxt = edge_prefetch(2, j + 1) if j + 1 < NT else None
                edge(2, j, pf)
                pf = nxt

    return nc


# ---------------------------------------------------------------------------
# Entry
# ---------------------------------------------------------------------------

def prepare_inputs(prep, inputs):
    x = np.asarray(inputs["x"], np.float32)
    W1a = fold_weights(np.asarray(inputs["W1"], np.float32),
                       np.asarray(inputs["att_src1"], np.float32),
                       np.asarray(inputs["att_dst1"], np.float32))
    W2a = fold_weights(np.asarray(inputs["W2"], np.float32),
                       np.asarray(inputs["att_src2"], np.float32),
                       np.asarray(inputs["att_dst2"], np.float32))
    b1 = np.asarray(inputs["b1"], np.float32)
    b2 = np.asarray(inputs["b2"], np.float32)
    Wm1 = np.asarray(inputs["Wm1"], np.float32)
    bm1 = np.asarray(inputs["bm1"], np.float32)
    Wm2 = np.asarray(inputs["Wm2"], np.float32)
    bm2 = np.asarray(inputs["bm2"], np.float32)

    oon = prep["old_of_new"]
    xp = np.zeros((NP, IN_DIM), np.float32)
    xp[oon >= 0] = x[oon[oon >= 0]]

    wm2a = np.concatenate([Wm2, bm2[None, :]], axis=0).astype(np.float16)
    shared = dict(
        w1a=W1a,
        w2a=np.concatenate([W2a[:128], W2a[128:]], axis=1).astype(np.float16),
        b1b=np.tile(b1[None, :], (128, 1)).astype(np.float32),
        b2b=np.tile(b2[None, :], (128, 1)).astype(np.float32),
        wm1=np.concatenate([Wm1[:128], Wm1[128:]], axis=1).astype(np.float16),
        bm1r=bm1[None, :].astype(np.float16),
        wm2a=wm2a,
        identf=np.eye(128, dtype=np.float16),
        iotaf=np.tile(np.arange(128, dtype=np.float16)[None, :], (128, 1)),
        poim=np.concatenate([np.zeros((127, 4), np.float16),
                             np.full((1, 4), POISON, np.float16)]),
        ones1=np.ones((1, 128), np.float16),
    )
    in_maps = []
    gbv, rkv = prep["gb"], prep["rows_k"]
    for c in range(NC):
        rows = np.concatenate([gbv[k] + c * rkv[k] + np.arange(rkv[k])
                               for k in range(len(rkv))])
        m = dict(shared)
        m["xT"] = np.ascontiguousarray(xp[rows].T)
        m["idxlo"] = prep["idx_lo"][c]
        m["idxhi"] = prep["idx_hi"][c]
        m["psel"] = prep["psel"][c]
        m["pselT"] = prep["pselT"][c]
        in_maps.append(m)
    return in_maps


def run(inputs, want_trace=False):
    prep = host_prep(np.asarray(inputs["edge_index"]))
    nc = build_bass(prep)
    nc.compile()
    in_maps = prepare_inputs(prep, inputs)
    res = run_bass_kernel_spmd(
        nc, in_maps, list(range(NC)), trace=want_trace)
    outs = np.concatenate([res.results[c]["out"] for c in range(NC)])
    outs = outs[prep["outperm"]]          # core-major -> global table rows
    oon = prep["old_of_new"]
    final = np.zeros((N, OUT_DIM), np.float32)
    valid = oon >= 0
    final[oon[valid]] = outs[valid]
    return final, res


def kernel(**inputs):
    out, _ = run(inputs)
    return out
